# revision 10
# baseline (speedup 1.0000x reference)
"""Trainium2 Bass kernel for nn_ConvolutionalCapsules_66477503808119.

Mathematical reduction of the reference:
  * The routing chain (layernorm -> cosine sim -> top-k -> ws iterations)
    feeds only softmax(ws, axis=6) where axis 6 has size 1, so the routing
    coefficients `a` are identically 1.0 and the whole chain is dead code.
  * Therefore s_j = sum_ic preds[b, ic] and, since conv is linear,
    s_j[b] = p4conv(sum_ic x[b, ic], w, IC * bias).
  * Output = squash_over_rotation(s_j):
        nsq   = sum_r s_j^2
        scale = nsq / ((1 + nsq) * (sqrt(nsq) + 1e-8))
              ~= sqrt(nsq) / (1 + nsq)          (verified: identical to fp32 rounding)
        out_r = scale * s_j_r

Sharding: pure data-parallel over batch, B=8 -> one batch element per core.

Per-core device kernel:
  1. DMA x[b] in as (128 part = (u2, c4=64), free = (v8, p=1024)), ic = u*8+v.
  2. VectorE reduce over v -> partial ic-sums written into a zero-padded
     (128, 34, 34) spatial plane (both u-halves kept separate).
  3. Conv as 9 shifted matmuls per (rotation r, spatial half): contraction
     K = (u, c4) = 128 with weights duplicated over u, which folds the
     remaining u-halves for free.  fp32r (1-pass fp22) matmuls.
  4. Squash on ScalarE/VectorE/GpSimd:  1/(1+nsq) = exp(-ln(1+nsq)).
  5. DMA out (128 part = o, free = (r, p)) -> (OC, OD, 4, H, W) per batch.
"""

import numpy as np

B, IC, ID, OC, OD = 8, 16, 16, 8, 16
KS, PAD, H, W = 3, 1, 32, 32
C4 = ID * 4          # 64 conv input channels
P = H * W            # 1024 spatial positions
O = OC * OD          # 128 output channels (pre-rotation)
N_CORES = 8

_CACHE = {}


def _build_nc():
    """Build + compile the single-core Bass program (shared by all 8 cores)."""
    if "nc" in _CACHE:
        return _CACHE["nc"]

    import concourse.mybir as mybir
    import concourse.tile as tile
    from concourse import bacc

    f32 = mybir.dt.float32
    f32r = mybir.dt.float32r
    ACT = mybir.ActivationFunctionType
    ALU = mybir.AluOpType

    nc = bacc.Bacc("TRN2", target_bir_lowering=False, debug=False)

    x_d = nc.dram_tensor("xin", (128, 8 * P), f32, kind="ExternalInput").ap()
    w_d = nc.dram_tensor("wt", (128, 4 * 9 * 128), f32r, kind="ExternalInput").ap()
    z_d = nc.dram_tensor("zeros", (128, 68), f32r, kind="ExternalInput").ap()
    b_d = nc.dram_tensor("bias16", (128, 1), f32, kind="ExternalInput").ap()
    out_d = nc.dram_tensor("out", (128, 4 * P), f32, kind="ExternalOutput").ap()

    x_src = x_d.rearrange("q (v p) -> q v p", v=8)
    out_dst = out_d.rearrange("o (r hh p) -> o r hh p", r=4, hh=2)

    with tile.TileContext(nc) as tc:
        with tc.tile_pool(name="cst", bufs=1) as cst, \
             tc.tile_pool(name="wrk", bufs=2) as wrk, \
             tc.tile_pool(name="ps", bufs=2, space="PSUM") as psp:

            xin = cst.tile([128, 8, P], f32)
            xpad = cst.tile([128, 34, 34], f32r)
            wt = cst.tile([128, 4, 9, 128], f32r)
            bias = cst.tile([128, 1], f32)

            nc.sync.dma_start(wt[:], w_d.rearrange("k (r t o) -> k r t o", r=4, t=9))
            nc.sync.dma_start(bias[:], b_d)
            # zero the conv padding border (memset can't write f32r; DMA can)
            nc.sync.dma_start(xpad[:, 0, :], z_d[:, 0:34])          # top row
            nc.sync.dma_start(xpad[:, 33, :], z_d[:, 34:68])        # bottom row
            nc.sync.dma_start(xpad[:, 1:33, 0], z_d[:, 0:32])       # left column
            nc.sync.dma_start(xpad[:, 1:33, 33], z_d[:, 0:32])      # right column

            # input DMA + ic partial-sum reduction, chunked for overlap
            NCH = 4
            pw = P // NCH            # 256 positions = 8 spatial rows per chunk
            nrows = pw // W
            for c in range(NCH):
                nc.sync.dma_start(
                    xin[:, :, c * pw:(c + 1) * pw], x_src[:, :, c * pw:(c + 1) * pw]
                )
                red_in = xin[:, :, c * pw:(c + 1) * pw].transpose([0, 2, 1])
                red_out = xpad[:, 1 + c * nrows:1 + (c + 1) * nrows, 1:33]
                with nc.allow_low_precision(reason="fp32r rounds only the final sum"):
                    nc.vector.tensor_reduce(red_out, red_in,
                                            axis=mybir.AxisListType.X, op=ALU.add)

            # conv + squash, per spatial half (16 output rows = 512 positions)
            for half in range(2):
                h0 = 16 * half
                psums = []
                for r in range(4):
                    ps = psp.tile([128, 512], f32, tag=f"ps{r}", name=f"ps_{half}_{r}")
                    t = 0
                    for a in range(3):
                        for bb in range(3):
                            rhs = xpad[:, h0 + a:h0 + a + 16, bb:bb + 32]
                            nc.tensor.matmul(
                                ps,
                                wt[:, r, t, :],
                                rhs,
                                start=(t == 0),
                                stop=(t == 8),
                            )
                            t += 1
                    psums.append(ps)

                # nsq = sum_r (psum_r + bias)^2
                sqs = []
                for r in range(4):
                    sq = wrk.tile([128, 512], f32, tag=f"sq{r}", name=f"sq_{half}_{r}")
                    nc.scalar.activation(sq, psums[r], ACT.Square, bias=bias[:, :],
                                         scale=1.0)
                    sqs.append(sq)
                n01 = wrk.tile([128, 512], f32, tag="n01", name=f"n01_{half}")
                nc.vector.tensor_add(n01, sqs[0], sqs[1])
                n23 = wrk.tile([128, 512], f32, tag="n23", name=f"n23_{half}")
                nc.gpsimd.tensor_tensor(n23, sqs[2], sqs[3], op=ALU.add)
                nsq = wrk.tile([128, 512], f32, tag="nsq", name=f"nsq_{half}")
                nc.vector.tensor_add(nsq, n01, n23)

                # scale = sqrt(nsq) / (1 + nsq);  1/(1+nsq) = exp(-ln(nsq+1))
                rootn = wrk.tile([128, 512], f32, tag="rootn", name=f"rootn_{half}")
                nc.scalar.activation(rootn, nsq, ACT.Sqrt)
                lnv = wrk.tile([128, 512], f32, tag="lnv", name=f"lnv_{half}")
                nc.scalar.activation(lnv, nsq, ACT.Ln, bias=1.0, scale=1.0)
                rinv = wrk.tile([128, 512], f32, tag="rinv", name=f"rinv_{half}")
                nc.scalar.activation(rinv, lnv, ACT.Exp, bias=0.0, scale=-1.0)
                sc = wrk.tile([128, 512], f32, tag="sc", name=f"sc_{half}")
                nc.vector.tensor_mul(sc, rootn, rinv)

                # out_r = (psum_r + bias) * scale
                for r in range(4):
                    ot = wrk.tile([128, 512], f32, tag=f"ot{r}", name=f"ot_{half}_{r}")
                    nc.vector.scalar_tensor_tensor(ot, psums[r], bias[:, :], sc,
                                                   op0=ALU.add, op1=ALU.mult)
                    nc.sync.dma_start(out_dst[:, r, half, :], ot)

    nc.compile()
    _CACHE["nc"] = nc
    return nc


def _prep_weights(conv_w, conv_b):
    """Host-side p4 filter transform -> lhsT tiles [(u,c4), (r, tap, o)]."""
    w = np.asarray(conv_w, dtype=np.float32)      # (O=128, ID=16, 4, 3, 3)
    tw = np.stack(
        [np.rot90(np.roll(w, r, axis=2), k=r, axes=(3, 4)) for r in range(4)],
        axis=1,
    )                                             # (O, r, i, s, a, b)
    # lhsT[(i,s), o] per (r, tap=(a,b)):
    wh = tw.transpose(1, 4, 5, 2, 3, 0).reshape(4, 9, C4, O)   # (r, tap, c4, o)
    wd = np.concatenate([wh, wh], axis=2)                      # duplicate over u
    w_dram = np.ascontiguousarray(
        wd.transpose(2, 0, 1, 3).reshape(128, 4 * 9 * 128), dtype=np.float32
    )
    bias16 = np.ascontiguousarray(
        (np.float32(IC) * np.asarray(conv_b, dtype=np.float32)).reshape(128, 1)
    )
    return w_dram, bias16


def make_in_maps(x, conv_w, conv_b):
    """Shard/lay out full inputs into per-core DRAM input maps."""
    x = np.asarray(x, dtype=np.float32)
    assert x.shape == (B, IC, ID, 4, H, W), x.shape
    w_dram, bias16 = _prep_weights(conv_w, conv_b)
    # (B, ic, c4, p) -> (B, u, c4, v, p) -> (B, 128, 8192)
    xr = x.reshape(B, 2, 8, C4, P).transpose(0, 1, 3, 2, 4).reshape(B, 128, 8 * P)
    xr = np.ascontiguousarray(xr)
    zeros = np.zeros((128, 68), dtype=np.float32)
    return [
        {"xin": xr[b], "wt": w_dram, "bias16": bias16, "zeros": zeros}
        for b in range(N_CORES)
    ]


def kernel(x, conv_w, conv_b, ln_gamma=None, ln_beta=None, k=None, ITER=None,
           **_unused):
    """Full-input, full-output entry point.  Shards batch over 8 cores."""
    from concourse.bass_utils import run_bass_kernel_spmd

    nc = _build_nc()
    in_maps = make_in_maps(x, conv_w, conv_b)
    res = run_bass_kernel_spmd(nc, in_maps, core_ids=list(range(N_CORES)))

    out = np.empty((B, OC, OD, 4, H, W), dtype=np.float32)
    for b in range(N_CORES):
        out[b] = res.results[b]["out"].reshape(O, 4, P).reshape(OC, OD, 4, H, W)
    return out
